# revision 1
# baseline (speedup 1.0000x reference)
"""Banded (Longformer-style) multi-head attention on 8 TRN2 NeuronCores.

Sharding: 16 heads are split 2-per-core (tensor parallel on H); every
core sees all 8192 tokens.  Compute dtype is bf16 (f32 accumulate in
PSUM); inputs are pre-cast/pre-transposed on the host (numerically
identical to casting on device, saves a full f32 pass over x).

Per-core kernel (single NEFF, software-pipelined emission so proj /
attention / out-proj overlap):
  1. DMA-transpose x slabs DRAM->SBUF to feature-major xT; project to
     qT,kT (feature-major [d, T]) and v (token-major, ones-augmented
     so the P@V matmul also produces the softmax denominator).
  2. j-major banded attention: for each 128-wide key tile j, one
     scores^T matmul [key,y x query-cols] against the <=5 query chunks
     in its band (K=64), exp on ScalarE without max-subtraction
     (scores are O(+-30), exact in f32), band-corner masking via
     affine_select on the idle GpSimd engine, then per-query-chunk
     P^T@V_aug accumulation (K=128) and a 1/den fixup on VectorE.
  3. partial output projection ctx_h @ Wo_h.T -> [8192, 1024] bf16.
The host sums the 8 partial outputs and adds the output bias (the
all-reduce step of tensor parallelism, done during the gather).
"""

import sys

sys.path.insert(0, "/opt/trn_rl_repo")

import numpy as np

import concourse.bass as bass
import concourse.mybir as mybir
import concourse.tile as tile
from concourse import bacc
from concourse.bass_utils import run_bass_kernel_spmd

F32 = mybir.dt.float32
BF16 = mybir.dt.bfloat16

B, S, D, E, H, HD = 2, 4096, 1024, 1024, 16, 64
W = 256                    # half window
T = B * S                  # 8192 flattened tokens
NCORES = 8
HPC = H // NCORES          # 2 heads per core
FQKV = 3 * HPC * HD        # 384 projected features per core
NT = T // 128              # 64 token chunks
CPS = S // 128             # 32 chunks per sequence
SLAB = 512                 # proj token slab
NSLAB = T // SLAB          # 16
VROW = 2 * (HD + 1)        # 130: [v_h0(64) | 1 | v_h1(64) | 1]


def _build_program():
    nc = bacc.Bacc(None, target_bir_lowering=False, debug=False)

    x_d = nc.dram_tensor("xbf", [T, D], BF16, kind="ExternalInput")
    wqkvT_d = nc.dram_tensor("wqkvT", [D, FQKV], BF16, kind="ExternalInput")
    bqkv_d = nc.dram_tensor("bqkv", [FQKV], F32, kind="ExternalInput")
    woT_d = nc.dram_tensor("woT", [HPC * HD, E], BF16, kind="ExternalInput")
    out_d = nc.dram_tensor("out_p", [T, E], BF16, kind="ExternalOutput")

    with tile.TileContext(nc) as tc:
        with (
            tc.tile_pool(name="const", bufs=1) as cpool,
            tc.tile_pool(name="big", bufs=1) as bigpool,
            tc.tile_pool(name="xtp", bufs=3) as xtp,
            tc.tile_pool(name="vio", bufs=2) as vio,
            tc.tile_pool(name="att", bufs=6) as att,
            tc.tile_pool(name="ptp", bufs=8) as ptp,
            tc.tile_pool(name="outsb", bufs=2) as outsb,
            tc.tile_pool(name="ps512", bufs=2, space="PSUM") as ps512,
            tc.tile_pool(name="spsum", bufs=2, space="PSUM") as spsum,
            tc.tile_pool(name="cpsum", bufs=2, space="PSUM") as cpsum,
        ):
            # ---- constants ----
            w_sb = cpool.tile([128, 8, FQKV], BF16, tag="w_sb")
            nc.sync.dma_start(
                w_sb[:], wqkvT_d[:].rearrange("(c p) f -> p c f", p=128))
            wo_sb = cpool.tile([128, E], BF16, tag="wo_sb")
            nc.sync.dma_start(wo_sb[:], woT_d[:])
            b_sb = cpool.tile([128, 3], F32, tag="b_sb")
            nc.sync.dma_start(b_sb[:], bqkv_d[:].rearrange("(a p) -> p a", p=128))

            # ---- persistent activations ----
            q_sb = bigpool.tile([128, T], BF16, tag="q_sb")
            k_sb = bigpool.tile([128, T], BF16, tag="k_sb")
            v_sb = bigpool.tile([128, NT, VROW], BF16, tag="v_sb")
            ctxT_sb = bigpool.tile([128, T], BF16, tag="ctxT_sb")
            # ones columns of the augmented V (cols 64 and 129 of each chunk)
            nc.vector.memset(v_sb[:, :, HD::HD + 1], 1.0)

            def proj_slab(t0, ntok):
                # ntok tokens starting at t0 (multiple of 128, <= 512)
                nck = ntok // 128
                xT = xtp.tile([128, 8, SLAB], BF16, tag="xT")
                nc.sync.dma_start_transpose(
                    xT[:, :, 0:ntok], x_d[t0:t0 + ntok, :])
                vT = xtp.tile([128, SLAB], BF16, tag="vT")
                for ft in range(3):
                    ps = ps512.tile([128, SLAB], F32, tag="ps512")
                    for c in range(8):
                        nc.tensor.matmul(
                            ps[:, 0:ntok], w_sb[:, c, ft * 128:(ft + 1) * 128],
                            xT[:, c, 0:ntok], start=(c == 0), stop=(c == 7))
                    dest = (q_sb, k_sb, None)[ft]
                    if dest is not None:
                        nc.vector.tensor_scalar_add(
                            dest[:, t0:t0 + ntok], ps[:, 0:ntok],
                            b_sb[:, ft:ft + 1])
                    else:
                        nc.vector.tensor_scalar_add(
                            vT[:, 0:ntok], ps[:, 0:ntok], b_sb[:, ft:ft + 1])
                # v: feature-major -> token-major (slab transpose), then
                # repack into the ones-augmented layout (one copy per head)
                vtmp = vio.tile([128, SLAB // 128, 128], BF16, tag="vtmp")
                nc.sync.dma_start_transpose(vtmp[:, 0:nck, :], vT[:, 0:ntok])
                for h in range(HPC):
                    nc.vector.tensor_copy(
                        v_sb[:, t0 // 128:t0 // 128 + nck,
                             h * (HD + 1):h * (HD + 1) + HD],
                        vtmp[:, 0:nck, h * HD:(h + 1) * HD])

            # j-major scoresT: st_j[y, b*128:(b+1)*128] = k_j^T q_{c}, where
            # c = j-2+b.  pt_j = exp(st_j/8) with band corners zeroed via
            # affine_select on GpSimd.
            pt_tiles = {}
            cn_state = {}

            def scores_j(seq, j, h):
                b_lo = max(0, 2 - j)
                b_hi = min(4, 2 + (CPS - 1) - j)
                gj = seq * CPS + j
                st = spsum.tile([128, 640], F32, tag="st")
                lo, hi = b_lo * 128, (b_hi + 1) * 128
                qcols = (seq * CPS + j - 2) * 128
                pieces = [(a, b) for (a, b) in [(lo, min(hi, 512)), (512, hi)]
                          if b > a]
                for (a, b) in pieces:
                    nc.tensor.matmul(
                        st[:, a:b],
                        k_sb[h * HD:(h + 1) * HD, gj * 128:(gj + 1) * 128],
                        q_sb[h * HD:(h + 1) * HD, qcols + a:qcols + b],
                        start=True, stop=True)
                pt = ptp.tile([128, 640], BF16, tag="pt")
                nc.scalar.activation(
                    pt[:, lo:hi], st[:, lo:hi],
                    mybir.ActivationFunctionType.Exp,
                    scale=float(1.0 / np.sqrt(HD)))
                if b_lo == 0:
                    # b=0 <-> chunk c=j-2, m=4: keep y <= t  (p <= f)
                    nc.gpsimd.affine_select(
                        out=pt[:, 0:128], in_=pt[:, 0:128],
                        compare_op=mybir.AluOpType.is_ge, fill=0.0, base=0,
                        pattern=[[1, 128]], channel_multiplier=-1)
                if b_hi == 4:
                    # b=4 <-> chunk c=j+2, m=0: keep y >= t  (p >= f)
                    nc.gpsimd.affine_select(
                        out=pt[:, 512:640], in_=pt[:, 512:640],
                        compare_op=mybir.AluOpType.is_ge, fill=0.0, base=0,
                        pattern=[[-1, 128]], channel_multiplier=1)
                pt_tiles[(seq, j, h)] = pt

            def attention_chunk(gc):
                seq, c = divmod(gc, CPS)
                qi, ci = divmod(gc, 2)
                m_lo = max(0, 2 - c)
                m_hi = min(4, CPS - 1 - c + 2)
                nm = m_hi - m_lo + 1
                if ci == 0:
                    cnq = att.tile([128, 2, 2, HD], BF16, tag="cn", name="cnq")
                    cn_state[qi] = cnq
                cn = cn_state[qi]
                for h in range(HPC):
                    ctx = cpsum.tile([128, HD + 1], F32, tag="ctx")
                    for mi, m in enumerate(range(m_lo, m_hi + 1)):
                        j = c - 2 + m
                        pt = pt_tiles[(seq, j, h)]
                        b = c - j + 2
                        nc.tensor.matmul(
                            ctx[:], pt[:, b * 128:(b + 1) * 128],
                            v_sb[:, seq * CPS + j,
                                 h * (HD + 1):(h + 1) * (HD + 1)],
                            start=(mi == 0), stop=(mi == nm - 1))
                    rec = att.tile([128, 1], F32, tag="rec")
                    nc.vector.reciprocal(rec[:], ctx[:, HD:HD + 1])
                    nc.vector.tensor_scalar_mul(cn[:, ci, h, :], ctx[:, 0:HD],
                                                rec[:])
                if ci == 1:
                    # 2-chunk batched transpose into feature-major ctxT
                    nc.sync.dma_start_transpose(
                        ctxT_sb[:, qi * 256:(qi + 1) * 256].rearrange(
                            "p (a b) -> p a b", a=2),
                        cn_state.pop(qi)[:].rearrange("p a b c -> p (a b c)"))

            def outproj_quad(qi):
                ob = outsb.tile([128, 4, E], BF16, tag="ob")
                for ci in range(4):
                    gc = qi * 4 + ci
                    for half in range(2):
                        op = ps512.tile([128, 512], F32, tag="ps512")
                        nc.tensor.matmul(
                            op[:], ctxT_sb[:, gc * 128:(gc + 1) * 128],
                            wo_sb[:, half * 512:(half + 1) * 512],
                            start=True, stop=True)
                        if (gc + half) % 2 == 0:
                            nc.scalar.activation(
                                ob[:, ci, half * 512:(half + 1) * 512], op[:],
                                mybir.ActivationFunctionType.Copy)
                        else:
                            nc.vector.tensor_copy(
                                ob[:, ci, half * 512:(half + 1) * 512], op[:])
                t0 = qi * 4 * 128
                nc.sync.dma_start(
                    out_d[t0:t0 + 512, :].rearrange("(c p) e -> p c e", p=128),
                    ob[:])

            # software-pipelined emission; smaller leading slabs so the
            # attention pipeline starts sooner
            widths = [128, 128, 256] + [SLAB] * ((T - 512) // SLAB)
            sc_done = [0] * B
            att_done = 0
            op_done = 0
            proj_chunks = 0
            for wd in widths:
                proj_slab(proj_chunks * 128, wd)
                proj_chunks += wd // 128
                for seq in range(B):
                    while (sc_done[seq] < CPS and
                           seq * CPS + min(sc_done[seq] + 2, CPS - 1)
                           < proj_chunks):
                        for h in range(HPC):
                            scores_j(seq, sc_done[seq], h)
                        sc_done[seq] += 1
                while att_done < NT:
                    seq, c = divmod(att_done, CPS)
                    if min(c + 2, CPS - 1) >= sc_done[seq]:
                        break
                    attention_chunk(att_done)
                    att_done += 1
                while (op_done + 1) * 4 <= att_done - 8:
                    outproj_quad(op_done)
                    op_done += 1
            while op_done * 4 < NT:
                outproj_quad(op_done)
                op_done += 1

    nc.compile()
    return nc


_NC_CACHE = None


def _get_program():
    global _NC_CACHE
    if _NC_CACHE is None:
        _NC_CACHE = _build_program()
    return _NC_CACHE


def make_core_inputs(x, Wqkv, bqkv, Wo):
    """Host-side shard prep: per-core reordered/transposed weight slices.
    bf16 is the on-device compute dtype; casting here (vs on-device) is
    numerically identical and saves a full f32 pass over x."""
    import ml_dtypes
    bf16 = ml_dtypes.bfloat16
    xbf = np.ascontiguousarray(x.reshape(T, D)).astype(bf16)
    in_maps = []
    for ci in range(NCORES):
        heads = [HPC * ci + i for i in range(HPC)]
        rows = []
        brows = []
        for comp in range(3):
            for h in heads:
                sl = slice(h * 3 * HD + comp * HD, h * 3 * HD + (comp + 1) * HD)
                rows.append(Wqkv[sl])
                brows.append(bqkv[sl])
        wq = np.ascontiguousarray(
            np.concatenate(rows, axis=0).T.astype(np.float32)).astype(bf16)
        bq = np.concatenate(brows).astype(np.float32)
        cols = np.concatenate([np.arange(h * HD, (h + 1) * HD) for h in heads])
        woT = np.ascontiguousarray(
            Wo[:, cols].T.astype(np.float32)).astype(bf16)
        in_maps.append({
            "xbf": xbf, "wqkvT": wq, "bqkv": bq, "woT": woT,
        })
    return in_maps


def _reference_numpy(x, padding_mask, Wqkv, bqkv, Wo, bo):
    """Exact fallback (only used if padding_mask is not all ones)."""
    NEG = -9e15
    Bx, Sx, Dx = x.shape
    Hh, hd, w = H, HD, W
    qkv = (x.reshape(-1, Dx) @ Wqkv.T + bqkv).reshape(Bx, Sx, Hh, 3, hd)
    q = np.transpose(qkv[..., 0, :], (0, 2, 1, 3))
    k = np.transpose(qkv[..., 1, :], (0, 2, 1, 3))
    v = np.transpose(qkv[..., 2, :], (0, 2, 1, 3))
    nb = Sx // w
    idx = (np.arange(nb) * w)[:, None] + np.arange(3 * w)[None, :]
    kp = np.pad(k, ((0, 0), (0, 0), (w, w), (0, 0)))
    vp = np.pad(v, ((0, 0), (0, 0), (w, w), (0, 0)))
    k_c = kp[:, :, idx, :]
    v_c = vp[:, :, idx, :]
    sc = np.einsum('bhnxd,bhnyd->bhnxy', q.reshape(Bx, Hh, nb, w, hd), k_c)
    x_i = np.arange(w)[:, None]
    j_i = x_i + np.arange(2 * w + 1)[None, :]
    band = sc[..., x_i, j_i]
    key_pos = np.arange(Sx).reshape(nb, w)[:, :, None] - w + np.arange(2 * w + 1)
    valid = (key_pos >= 0) & (key_pos < Sx)
    km = padding_mask[:, np.clip(key_pos, 0, Sx - 1)] != 0
    m = valid[None, None] & km[:, None]
    band = np.where(m, band, NEG)
    band = band / np.sqrt(hd)
    band = band - band.max(axis=-1, keepdims=True)
    e = np.exp(band)
    attn = e / e.sum(axis=-1, keepdims=True)
    attn = np.where(m, attn, 0.0)
    a3 = np.zeros_like(sc)
    a3[..., x_i, j_i] = attn
    ctx = np.einsum('bhnxy,bhnyd->bhnxd', a3, v_c).reshape(Bx, Hh, Sx, hd)
    out = np.transpose(ctx, (0, 2, 1, 3)).reshape(Bx, Sx, Hh * hd)
    return (out @ Wo.T + bo).astype(np.float32)


def kernel(x, padding_mask, Wqkv, bqkv, Wo, bo):
    x = np.asarray(x)
    padding_mask = np.asarray(padding_mask)
    Wqkv = np.asarray(Wqkv, dtype=np.float32)
    bqkv = np.asarray(bqkv, dtype=np.float32)
    Wo = np.asarray(Wo, dtype=np.float32)
    bo = np.asarray(bo, dtype=np.float32)
    if not np.all(padding_mask != 0):
        return _reference_numpy(x.astype(np.float32), padding_mask,
                                Wqkv, bqkv, Wo, bo)
    nc = _get_program()
    in_maps = make_core_inputs(x, Wqkv, bqkv, Wo)
    res = run_bass_kernel_spmd(nc, in_maps, core_ids=list(range(NCORES)))
    acc = np.zeros((T, E), np.float32)
    for ci in range(NCORES):
        acc += np.asarray(res.results[ci]["out_p"]).astype(np.float32)
    acc += bo[None, :]
    return acc.reshape(B, S, E)

